# revision 15
# baseline (speedup 1.0000x reference)
"""Distributed sparse-attention kernel for 8 TRN2 NeuronCores.

Sharding: Megatron-style head parallelism. Core c owns heads [4c, 4c+4):
Wq/Wk/Wv column-parallel (rows of the [H*DH, D] weights), Wo row-parallel
(columns of [D, H*DH]). Each core computes a partial output
out_c = Wo_c @ ctx_c over its heads; the host sums the 8 partials.

Device layout choices (no on-chip transposes anywhere):
  h_q, h_k   : [dh, len] fp16  (proj psum M=dh-chunk, N=len)
  h_vT       : [k, dh+1] bf16  (proj psum M=k-chunk, N=dh; ones column
               appended so the ctx matmul also emits the softmax denom)
  scores/exp : [k, q]  (pb+mask bias added on DVE, exp on ACT; no
               max-subtraction: scores are O(50) so exp fits fp32/bf16
               range, masked entries are -1e30 -> exp underflows to 0)
  ctx        : psum [65, q] f32; row 64 = sum_k exp  (denominator)
  normalize  : DVE reciprocal_approx_fast + gpsimd partition_broadcast
               + DVE mul -> ctxn bf16

Precision: Q/K path (projections + scores) in fp16 — scores feed exp(),
so absolute score error must stay ~1e-2; fp16 keeps it ~5e-3 while bf16
would give ~5e-2. V/ctx/output path errors only enter linearly, so bf16
is fine there (exp values overflow fp16 range, hence bf16 anyway).
V-projection packs two k-chunk accumulations per PSUM bank: the first
matmul's start=True clears has_written for the whole bank, the partner
k-chunk then starts with start=False and overwrites-where-unset.
"""

import sys

for _p in ("/opt/trn_rl_repo",):
    if _p not in sys.path:
        sys.path.insert(0, _p)

from contextlib import ExitStack

import numpy as np
import ml_dtypes

import concourse.bass as bass
import concourse.mybir as mybir
import concourse.tile as tile
from concourse import bacc
from concourse.bass_utils import run_bass_kernel_spmd

B, D, H, DH, LQ, LK = 2, 2048, 32, 64, 1024, 1024
NCORES = 8
HC = H // NCORES          # heads per core = 4
MR = HC * DH              # per-core model rows = 256
NEG = -1e30

DC = D // 128             # 16 d-chunks
NKC = LK // 128           # 8 k-chunks
NQB = LQ // 512           # 2 q blocks
NMC = MR // 128           # 2 dh-chunks
NOC = D // 128            # 16 output-row chunks

F32 = mybir.dt.float32
F32R = mybir.dt.float32r
F16 = mybir.dt.float16
BF16 = mybir.dt.bfloat16

CFG = dict(
    dt_hid=F16,    # HBM hidden_q / hidden_kv (Q/K-path matmul operand)
    dt_w=F16,      # HBM Wq/Wk/Wv
    dt_wo=BF16,    # HBM Wo
    dt_pbm=BF16,   # HBM combined position_bias + mask bias
    dt_out=F32,    # HBM partial output
    dt_qk=F16,     # SBUF h_q / h_k
    dt_v=BF16,     # SBUF h_vT / exp / ctxn
    v_pair=True,   # pack two V k-chunk accumulations per PSUM bank
    fast_recip=False,  # reciprocal_approx_fast vs exact reciprocal
)

_NP = {F32: np.float32, F32R: np.float32, F16: np.float16,
       BF16: ml_dtypes.bfloat16}


def build_nc():
    dt_hid, dt_w, dt_wo = CFG["dt_hid"], CFG["dt_w"], CFG["dt_wo"]
    dt_pbm, dt_out = CFG["dt_pbm"], CFG["dt_out"]
    dt_qk, dt_v = CFG["dt_qk"], CFG["dt_v"]

    nc = bacc.Bacc("TRN2", target_bir_lowering=False, debug=False,
                   num_devices=NCORES)
    hq_e = nc.declare_dram_parameter("hq", [B, D, LQ], dt_hid, False)
    hkv_e = nc.declare_dram_parameter("hkv", [B, D, LK], dt_hid, False)
    pbm_e = nc.declare_dram_parameter("pbm", [B, HC, LK, LQ], dt_pbm, False)
    wqt_e = nc.declare_dram_parameter("wqt", [D, MR], dt_w, False)
    wkt_e = nc.declare_dram_parameter("wkt", [D, MR], dt_w, False)
    wvt_e = nc.declare_dram_parameter("wvt", [D, MR], dt_w, False)
    wot_e = nc.declare_dram_parameter("wot", [MR, D], dt_wo, False)
    out_e = nc.declare_dram_parameter("out", [B, D, LQ], dt_out, True)
    hq_a, hkv_a, pbm_a, out_a = hq_e.ap(), hkv_e.ap(), pbm_e.ap(), out_e.ap()

    mm = nc.tensor.matmul
    Exp = mybir.ActivationFunctionType.Exp

    with tile.TileContext(nc) as tc, ExitStack() as ctx:
        wp = ctx.enter_context(tc.tile_pool(name="w", bufs=1))
        hidp = ctx.enter_context(tc.tile_pool(name="hid", bufs=4))
        sbp = ctx.enter_context(tc.tile_pool(name="sb", bufs=1))
        pbmp = ctx.enter_context(tc.tile_pool(name="pbm", bufs=4))
        tmpp = ctx.enter_context(tc.tile_pool(name="tmp", bufs=4))
        psp = ctx.enter_context(tc.tile_pool(name="ps", bufs=4, space="PSUM"))

        # --- persistent weights (emission order = DMA priority order) ----
        wk_sb, wv_sb, wq_sb = [], [], []
        for nm, ap_, lst in (("wk", wkt_e.ap(), wk_sb),
                             ("wv", wvt_e.ap(), wv_sb)):
            for dc in range(DC):
                t = wp.tile([128, MR], dt_w, tag=f"{nm}{dc}", name=f"{nm}{dc}")
                nc.sync.dma_start(t[:, :], ap_[dc * 128:(dc + 1) * 128, :])
                lst.append(t)
        ones1 = wp.tile([128, 1], F32, tag="ones1", name="ones1")
        nc.gpsimd.memset(ones1[:, :], 1.0)

        def load_wq():
            for dc in range(DC):
                t = wp.tile([128, MR], dt_w, tag=f"wq{dc}", name=f"wq{dc}")
                nc.sync.dma_start(t[:, :], wqt_e.ap()[dc * 128:(dc + 1) * 128, :])
                wq_sb.append(t)

        wo_sb = []

        def load_wo():
            for c in range(NMC):
                t = wp.tile([128, D], dt_wo, tag=f"wo{c}", name=f"wo{c}")
                nc.sync.dma_start(t[:, :], wot_e.ap()[c * 128:(c + 1) * 128, :])
                wo_sb.append(t)

        for b in range(B):
            # --- pass A: K proj (4 banks) + V proj, 2 k-chunks per bank --
            pk = [psp.tile([128, 512], F32, tag="big", name=f"pk{b}_{i}")
                  for i in range(4)]
            pv = [psp.tile([128, 512], F32, tag="pv", name=f"pv{b}_{i}")
                  for i in range(4)]
            hvT = [sbp.tile([128, HC * (DH + 1)], dt_v, tag=f"hv{kc}",
                            name=f"hv{b}_{kc}") for kc in range(NKC)]
            hk_sb = [sbp.tile([128, LK], dt_qk, tag=f"hk{mc}",
                              name=f"hk{b}_{mc}") for mc in range(NMC)]
            hq_sb = [sbp.tile([128, LQ], dt_qk, tag=f"hqs{mc}",
                              name=f"hqs{b}_{mc}") for mc in range(NMC)]

            v_pair = CFG["v_pair"]
            nv_a = NKC if v_pair else NKC // 2

            def evac_hvT(kc, src):
                for h in range(HC):
                    nc.vector.tensor_copy(
                        hvT[kc][:, h * (DH + 1):h * (DH + 1) + DH],
                        src[:, h * DH:(h + 1) * DH])
                    nc.vector.tensor_copy(
                        hvT[kc][:, h * (DH + 1) + DH:(h + 1) * (DH + 1)],
                        ones1[:, :])

            for dc in range(DC):
                hkv_t = hidp.tile([128, LK], dt_hid, tag="hkv",
                                  name=f"hkvA{b}_{dc}")
                nc.sync.dma_start(hkv_t[:, :],
                                  hkv_a[b, dc * 128:(dc + 1) * 128, :])
                for mc in range(NMC):
                    for kb in range(2):
                        mm(pk[mc * 2 + kb][:, :],
                           wk_sb[dc][:, mc * 128:(mc + 1) * 128],
                           hkv_t[:, kb * 512:(kb + 1) * 512],
                           start=dc == 0, stop=dc == DC - 1)
                for kc in range(nv_a):
                    if v_pair:
                        # two k-chunks share a psum bank; the first matmul
                        # clears the bank, the last one closes the group
                        mm(pv[kc // 2][:, (kc % 2) * MR:(kc % 2) * MR + MR],
                           hkv_t[:, kc * 128:(kc + 1) * 128],
                           wv_sb[dc][:, :],
                           start=(dc == 0 and kc % 2 == 0),
                           stop=(dc == DC - 1 and kc % 2 == 1))
                    else:
                        mm(pv[kc][:, 0:MR],
                           hkv_t[:, kc * 128:(kc + 1) * 128],
                           wv_sb[dc][:, :],
                           start=dc == 0, stop=dc == DC - 1)
            for mc in range(NMC):
                for kb in range(2):
                    nc.vector.tensor_copy(hk_sb[mc][:, kb * 512:(kb + 1) * 512],
                                          pk[mc * 2 + kb][:, :])
            for kc in range(nv_a):
                if v_pair:
                    evac_hvT(kc, pv[kc // 2][:, (kc % 2) * MR:
                                             (kc % 2) * MR + MR])
                else:
                    evac_hvT(kc, pv[kc][:, 0:MR])

            # --- pass B: Q proj (+ remaining V k-chunks if not v_pair) ----
            if b == 0:
                load_wq()
            pq = [psp.tile([128, 512], F32, tag="big", name=f"pq{b}_{i}")
                  for i in range(4)]
            pv2 = None
            if not v_pair:
                pv2 = [psp.tile([128, MR], F32, tag="pv", name=f"pv2{b}_{i}")
                       for i in range(4)]
            for dc in range(DC):
                hq_t = hidp.tile([128, LQ], dt_hid, tag="hq",
                                 name=f"hqB{b}_{dc}")
                nc.sync.dma_start(hq_t[:, :],
                                  hq_a[b, dc * 128:(dc + 1) * 128, :])
                for mc in range(NMC):
                    for qb in range(2):
                        mm(pq[mc * 2 + qb][:, :],
                           wq_sb[dc][:, mc * 128:(mc + 1) * 128],
                           hq_t[:, qb * 512:(qb + 1) * 512],
                           start=dc == 0, stop=dc == DC - 1)
                if not v_pair:
                    hkv_t = hidp.tile([128, LK], dt_hid, tag="hkv",
                                      name=f"hkvB{b}_{dc}")
                    nc.sync.dma_start(hkv_t[:, :],
                                      hkv_a[b, dc * 128:(dc + 1) * 128, :])
                    for kc in range(NKC // 2, NKC):
                        mm(pv2[kc - NKC // 2][:, 0:MR],
                           hkv_t[:, kc * 128:(kc + 1) * 128],
                           wv_sb[dc][:, :],
                           start=dc == 0, stop=dc == DC - 1)
            for mc in range(NMC):
                for qb in range(2):
                    nc.vector.tensor_copy(hq_sb[mc][:, qb * 512:(qb + 1) * 512],
                                          pq[mc * 2 + qb][:, :])
            if not v_pair:
                for kc in range(NKC // 2, NKC):
                    evac_hvT(kc, pv2[kc - NKC // 2][:, 0:MR])

            # --- attention, head pairs ------------------------------------
            ctxn = [sbp.tile([128, LQ], dt_v, tag=f"ctxn{c}",
                             name=f"ctxn{b}_{c}") for c in range(NMC)]
            for hp in range(HC // 2):
                hc = hp
                heads = (2 * hp, 2 * hp + 1)
                pctx = {}
                for h in heads:
                    for qb in range(NQB):
                        pctx[(h, qb)] = psp.tile([DH + 1, 512], F32, tag="pv",
                                                 name=f"pctx{b}_{h}_{qb}")
                for kc in range(NKC):
                    pbm_t = {}
                    for h in heads:
                        pbm_t[h] = pbmp.tile([128, LQ], dt_pbm, tag="pbm",
                                             name=f"pbm{b}_{h}_{kc}")
                        nc.sync.dma_start(
                            pbm_t[h][:, :],
                            pbm_a[b, h, kc * 128:(kc + 1) * 128, :])
                    for qb in range(NQB):
                        ps_t, ex_t = {}, {}
                        for h in heads:
                            po = (h % 2) * 64
                            ps_t[h] = psp.tile([128, 512], F32, tag="big",
                                               name=f"ps{b}_{h}_{kc}_{qb}")
                            mm(ps_t[h][:, :],
                               hk_sb[hc][po:po + 64, kc * 128:(kc + 1) * 128],
                               hq_sb[hc][po:po + 64, qb * 512:(qb + 1) * 512],
                               start=True, stop=True)
                        for h in heads:
                            tmp_t = tmpp.tile([128, 512], F32, tag="tmp",
                                              name=f"tm{b}_{h}_{kc}_{qb}")
                            nc.vector.tensor_add(
                                tmp_t[:, :], ps_t[h][:, :],
                                pbm_t[h][:, qb * 512:(qb + 1) * 512])
                            ex_t[h] = tmpp.tile([128, 512], dt_v, tag="exp",
                                                name=f"ex{b}_{h}_{kc}_{qb}")
                            nc.scalar.activation(ex_t[h][:, :], tmp_t[:, :], Exp)
                        for h in heads:
                            mm(pctx[(h, qb)][:, :],
                               hvT[kc][:, h * (DH + 1):(h + 1) * (DH + 1)],
                               ex_t[h][:, :],
                               start=kc == 0, stop=kc == NKC - 1)
                # normalize: ctx[0:64] * (1 / ctx[64])
                for h in heads:
                    po = (h % 2) * 64
                    for qb in range(NQB):
                        rc = tmpp.tile([1, 512], F32, tag="rc",
                                       name=f"rc{b}_{h}_{qb}")
                        if CFG["fast_recip"]:
                            nc.vector.reciprocal_approx_fast(
                                rc[:, :], pctx[(h, qb)][DH:DH + 1, :])
                        else:
                            nc.vector.reciprocal(
                                rc[:, :], pctx[(h, qb)][DH:DH + 1, :])
                        bc = tmpp.tile([64, 512], F32, tag="bc",
                                       name=f"bc{b}_{h}_{qb}")
                        nc.gpsimd.partition_broadcast(bc[:, :], rc[:, :])
                        nc.vector.tensor_mul(
                            ctxn[hc][po:po + 64, qb * 512:(qb + 1) * 512],
                            pctx[(h, qb)][0:DH, :], bc[:, :])

            # --- output projection ----------------------------------------
            if b == 0:
                load_wo()
            for oc in range(NOC):
                for qb in range(NQB):
                    po_t = psp.tile([128, 512], F32, tag="big",
                                    name=f"po{b}_{oc}_{qb}")
                    for c in range(NMC):
                        mm(po_t[:, :],
                           wo_sb[c][:, oc * 128:(oc + 1) * 128],
                           ctxn[c][:, qb * 512:(qb + 1) * 512],
                           start=c == 0, stop=c == NMC - 1)
                    osb = tmpp.tile([128, 512], dt_out, tag="osb",
                                    name=f"osb{b}_{oc}_{qb}")
                    nc.vector.tensor_copy(osb[:, :], po_t[:, :])
                    nc.sync.dma_start(
                        out_a[b, oc * 128:(oc + 1) * 128,
                              qb * 512:(qb + 1) * 512],
                        osb[:, :])

    nc.compile()
    return nc


_NC_CACHE = None


def _get_nc():
    global _NC_CACHE
    if _NC_CACHE is None:
        _NC_CACHE = build_nc()
    return _NC_CACHE


def make_in_maps(hidden_q, hidden_kv, mask, position_bias, Wq, Wk, Wv, Wo):
    np_hid = _NP[CFG["dt_hid"]]
    np_w = _NP[CFG["dt_w"]]
    np_wo = _NP[CFG["dt_wo"]]
    np_pbm = _NP[CFG["dt_pbm"]]
    hidden_q = np.asarray(hidden_q, np.float32)
    hidden_kv = np.asarray(hidden_kv, np.float32)
    mask = np.asarray(mask)
    position_bias = np.asarray(position_bias, np.float32)
    Wq, Wk, Wv, Wo = (np.asarray(w, np.float32) for w in (Wq, Wk, Wv, Wo))

    maskb = np.where(mask != 0, np.float32(0), np.float32(NEG))  # [B, LK, LQ]
    hq = hidden_q.astype(np_hid)
    hkv = hidden_kv.astype(np_hid)
    in_maps = []
    for c in range(NCORES):
        hs = slice(c * HC, (c + 1) * HC)
        rs = slice(c * MR, (c + 1) * MR)
        pbm = (position_bias[hs][None] + maskb[:, None]).astype(np_pbm)
        in_maps.append({
            "hq": hq,
            "hkv": hkv,
            "pbm": pbm,
            "wqt": np.ascontiguousarray(Wq[rs].T).astype(np_w),
            "wkt": np.ascontiguousarray(Wk[rs].T).astype(np_w),
            "wvt": np.ascontiguousarray(Wv[rs].T).astype(np_w),
            "wot": np.ascontiguousarray(Wo[:, rs].T).astype(np_wo),
        })
    return in_maps


def run(in_maps, trace=False):
    nc = _get_nc()
    return run_bass_kernel_spmd(nc, in_maps, core_ids=list(range(NCORES)),
                                trace=trace)


def kernel(hidden_q, hidden_kv, mask, position_bias, Wq, Wk, Wv, Wo):
    in_maps = make_in_maps(hidden_q, hidden_kv, mask, position_bias,
                           Wq, Wk, Wv, Wo)
    res = run(in_maps, trace=False)
    acc = np.zeros((B, D, LQ), np.float32)
    for r in res.results:
        acc += np.asarray(r["out"], dtype=np.float32)
    return acc


# revision 24
# speedup vs baseline: 1.1517x; 1.1517x over previous
"""Distributed sparse-attention kernel for 8 TRN2 NeuronCores.

Sharding: Megatron-style head parallelism. Core c owns heads [4c, 4c+4):
Wq/Wk/Wv column-parallel (rows of the [H*DH, D] weights), Wo row-parallel
(columns of [D, H*DH]). Each core computes a partial output
out_c = Wo_c @ ctx_c over its heads; the host sums the 8 partials.

Device layout choices (no on-chip transposes anywhere):
  h_q, h_k   : [dh, len] fp16  (proj psum M=dh-chunk, N=len)
  h_vT       : [k, dh+1] bf16  (proj psum M=k-chunk, N=dh; ones column
               appended so the ctx matmul also emits the softmax denom)
  scores     : [k, q] psum; pb+mask bias accumulated into the same psum
               bank by an identity-matmul (out += I.T @ pbm), exp on ACT
               straight from psum. No max-subtraction: scores are O(50)
               so exp fits fp32/bf16 range, masked entries are -1e30 ->
               exp underflows to exact 0.
  ctx        : psum [65, q] f32; row 64 = sum_k exp  (denominator)
  normalize  : reciprocal batched per head-pair on DVE + gpsimd
               partition_broadcast + DVE mul -> ctxn bf16

Precision: Q/K path (projections + scores) in fp16 — scores feed exp(),
so absolute score error must stay ~1e-2; fp16 keeps it ~5e-3 while bf16
would give ~5e-2. V/ctx/output path errors only enter linearly, so bf16
is fine there (exp values overflow fp16 range, hence bf16 anyway).
V-projection packs two k-chunk accumulations per PSUM bank: the first
matmul's start=True clears has_written for the whole bank, the partner
k-chunk then starts with start=False and overwrites-where-unset.
(Known-broken on HW, avoid: reciprocal_approx_fast — returns garbage.)
"""

import sys

for _p in ("/opt/trn_rl_repo",):
    if _p not in sys.path:
        sys.path.insert(0, _p)

from contextlib import ExitStack

import numpy as np
import ml_dtypes

import concourse.bass as bass
import concourse.mybir as mybir
import concourse.tile as tile
from concourse import bacc
from concourse.bass_utils import run_bass_kernel_spmd

B, D, H, DH, LQ, LK = 2, 2048, 32, 64, 1024, 1024
NCORES = 8
HC = H // NCORES          # heads per core = 4
MR = HC * DH              # per-core model rows = 256
NEG = -1e30

DC = D // 128             # 16 d-chunks
NKC = LK // 128           # 8 k-chunks
NQB = LQ // 512           # 2 q blocks
NMC = MR // 128           # 2 dh-chunks
NOC = D // 128            # 16 output-row chunks

F32 = mybir.dt.float32
F32R = mybir.dt.float32r
F16 = mybir.dt.float16
BF16 = mybir.dt.bfloat16

USE_IDENT_ADD = True

CFG = dict(
    dt_hid=F16,    # HBM hidden_q / hidden_kv (Q/K-path matmul operand)
    dt_w=F16,      # HBM Wq/Wk/Wv
    dt_wo=BF16,    # HBM Wo
    dt_pbm=BF16,   # HBM combined position_bias + mask bias
    dt_out=BF16,   # HBM partial output
    dt_qk=F16,     # SBUF h_q / h_k
    dt_v=BF16,     # SBUF h_vT / exp / ctxn
)

_NP = {F32: np.float32, F32R: np.float32, F16: np.float16,
       BF16: ml_dtypes.bfloat16}


def build_nc():
    dt_hid, dt_w, dt_wo = CFG["dt_hid"], CFG["dt_w"], CFG["dt_wo"]
    dt_pbm, dt_out = CFG["dt_pbm"], CFG["dt_out"]
    dt_qk, dt_v = CFG["dt_qk"], CFG["dt_v"]

    nc = bacc.Bacc("TRN2", target_bir_lowering=False, debug=False,
                   num_devices=NCORES)
    hq_e = nc.declare_dram_parameter("hq", [B, D, LQ], dt_hid, False)
    hkv_e = nc.declare_dram_parameter("hkv", [B, D, LK], dt_hid, False)
    pbm_e = nc.declare_dram_parameter("pbm", [B, HC, LK, LQ], dt_pbm, False)
    wqt_e = nc.declare_dram_parameter("wqt", [D, MR], dt_w, False)
    wkt_e = nc.declare_dram_parameter("wkt", [D, MR], dt_w, False)
    wvt_e = nc.declare_dram_parameter("wvt", [D, MR], dt_w, False)
    wot_e = nc.declare_dram_parameter("wot", [MR, D], dt_wo, False)
    id_e = nc.declare_dram_parameter("ident", [128, 128], dt_pbm, False)
    out_e = nc.declare_dram_parameter("out", [B, D, LQ], dt_out, True)
    hq_a, hkv_a, pbm_a, out_a = hq_e.ap(), hkv_e.ap(), pbm_e.ap(), out_e.ap()

    mm = nc.tensor.matmul
    Exp = mybir.ActivationFunctionType.Exp
    Ln = mybir.ActivationFunctionType.Ln

    with tile.TileContext(nc) as tc, ExitStack() as ctx:
        wp = ctx.enter_context(tc.tile_pool(name="w", bufs=1))
        hidp = ctx.enter_context(tc.tile_pool(name="hid", bufs=3))
        sbp = ctx.enter_context(tc.tile_pool(name="sb", bufs=1))
        pbmp = ctx.enter_context(tc.tile_pool(name="pbm", bufs=3))
        tmpp = ctx.enter_context(tc.tile_pool(name="tmp", bufs=4))
        psp = ctx.enter_context(tc.tile_pool(name="ps", bufs=4, space="PSUM"))

        # --- persistent weights, 4 d-chunks per DMA ----------------------
        # wX_sb[dc] are AP views into [128, 4*MR] tiles loaded with the
        # rearranged [(4c p) m -> p (4c m)] pattern.
        def load_w4(nm, ap_):
            views = []
            for g in range(DC // 4):
                t = wp.tile([128, 4 * MR], dt_w, tag=f"{nm}{g}", name=f"{nm}{g}")
                src = ap_[g * 512:(g + 1) * 512, :].rearrange(
                    "(c p) m -> p c m", c=4)
                nc.sync.dma_start(t[:, :].rearrange("p (c m) -> p c m", c=4),
                                  src)
                for i in range(4):
                    views.append(t[:, i * MR:(i + 1) * MR])
            return views

        wk_sb = load_w4("wk", wkt_e.ap())
        wv_sb = load_w4("wv", wvt_e.ap())
        ident = wp.tile([128, 128], dt_pbm, tag="ident", name="ident")
        nc.sync.dma_start(ident[:, :], id_e.ap()[:, :])
        ones1 = wp.tile([128, 1], F32, tag="ones1", name="ones1")
        nc.gpsimd.memset(ones1[:, :], 1.0)

        wq_sb, wo_sb = [], []

        def load_wq():
            wq_sb.extend(load_w4("wq", wqt_e.ap()))

        def load_wo():
            for c in range(NMC):
                t = wp.tile([128, D], dt_wo, tag=f"wo{c}", name=f"wo{c}")
                nc.sync.dma_start(t[:, :], wot_e.ap()[c * 128:(c + 1) * 128, :])
                wo_sb.append(t)

        for b in range(B):
            # --- pass A: K proj (4 banks) + V proj 2 k-chunks/bank (4) ---
            pk = [psp.tile([128, 512], F32, tag="big", name=f"pk{b}_{i}")
                  for i in range(4)]
            pv = [psp.tile([128, 512], F32, tag="pv", name=f"pv{b}_{i}")
                  for i in range(4)]
            hvT = [sbp.tile([128, HC * (DH + 1)], dt_v, tag=f"hv{kc}",
                            name=f"hv{b}_{kc}") for kc in range(NKC)]
            hk_sb = [sbp.tile([128, LK], dt_qk, tag=f"hk{mc}",
                              name=f"hk{b}_{mc}") for mc in range(NMC)]
            hq_sb = [sbp.tile([128, LQ], dt_qk, tag=f"hqs{mc}",
                              name=f"hqs{b}_{mc}") for mc in range(NMC)]

            for dc2 in range(0, DC, 2):
                hkv_t = hidp.tile([128, 2 * LK], dt_hid, tag="hkv",
                                  name=f"hkvA{b}_{dc2}")
                nc.sync.dma_start(
                    hkv_t[:, :].rearrange("p (c q) -> p c q", c=2),
                    hkv_a[b, dc2 * 128:(dc2 + 2) * 128, :].rearrange(
                        "(c p) q -> p c q", c=2))
                for i in range(2):
                    dc = dc2 + i
                    hv = hkv_t[:, i * LK:(i + 1) * LK]
                    for mc in range(NMC):
                        for kb in range(2):
                            mm(pk[mc * 2 + kb][:, :],
                               wk_sb[dc][:, mc * 128:(mc + 1) * 128],
                               hv[:, kb * 512:(kb + 1) * 512],
                               start=dc == 0, stop=dc == DC - 1)
                    for kc in range(NKC):
                        # two k-chunks share a psum bank; the first matmul
                        # clears the bank, the last one closes the group
                        mm(pv[kc // 2][:, (kc % 2) * MR:(kc % 2) * MR + MR],
                           hv[:, kc * 128:(kc + 1) * 128],
                           wv_sb[dc][:, :],
                           start=(dc == 0 and kc % 2 == 0),
                           stop=(dc == DC - 1 and kc % 2 == 1))
            for mc in range(NMC):
                for kb in range(2):
                    nc.vector.tensor_copy(hk_sb[mc][:, kb * 512:(kb + 1) * 512],
                                          pk[mc * 2 + kb][:, :])
            for kc in range(NKC):
                src = pv[kc // 2][:, (kc % 2) * MR:(kc % 2) * MR + MR]
                for h in range(HC):
                    nc.vector.tensor_copy(
                        hvT[kc][:, h * (DH + 1):h * (DH + 1) + DH],
                        src[:, h * DH:(h + 1) * DH])
                    nc.vector.tensor_copy(
                        hvT[kc][:, h * (DH + 1) + DH:(h + 1) * (DH + 1)],
                        ones1[:, :])

            # --- pass B: Q proj ------------------------------------------
            if b == 0:
                load_wq()
            pq = [psp.tile([128, 512], F32, tag="big", name=f"pq{b}_{i}")
                  for i in range(4)]
            for dc2 in range(0, DC, 2):
                hq_t = hidp.tile([128, 2 * LQ], dt_hid, tag="hq",
                                 name=f"hqB{b}_{dc2}")
                nc.sync.dma_start(
                    hq_t[:, :].rearrange("p (c q) -> p c q", c=2),
                    hq_a[b, dc2 * 128:(dc2 + 2) * 128, :].rearrange(
                        "(c p) q -> p c q", c=2))
                for i in range(2):
                    dc = dc2 + i
                    hv = hq_t[:, i * LQ:(i + 1) * LQ]
                    for mc in range(NMC):
                        for qb in range(2):
                            mm(pq[mc * 2 + qb][:, :],
                               wq_sb[dc][:, mc * 128:(mc + 1) * 128],
                               hv[:, qb * 512:(qb + 1) * 512],
                               start=dc == 0, stop=dc == DC - 1)
            for mc in range(NMC):
                for qb in range(2):
                    nc.vector.tensor_copy(hq_sb[mc][:, qb * 512:(qb + 1) * 512],
                                          pq[mc * 2 + qb][:, :])

            # --- attention, head pairs ------------------------------------
            ctxn = [sbp.tile([128, LQ], dt_v, tag=f"ctxn{c}",
                             name=f"ctxn{b}_{c}") for c in range(NMC)]
            for hp in range(HC // 2):
                hc = hp
                heads = (2 * hp, 2 * hp + 1)
                pctx = {}
                for h in heads:
                    for qb in range(NQB):
                        pctx[(h, qb)] = psp.tile([DH + 1, 512], F32, tag="pv",
                                                 name=f"pctx{b}_{h}_{qb}")
                for kc2 in range(0, NKC, 2):
                    pbm_t = {}
                    for h in heads:
                        pbm_t[h] = pbmp.tile([128, 2 * LQ], dt_pbm, tag="pbm",
                                             name=f"pbm{b}_{h}_{kc2}")
                        nc.sync.dma_start(
                            pbm_t[h][:, :].rearrange("p (c q) -> p c q", c=2),
                            pbm_a[b, h, kc2 * 128:(kc2 + 2) * 128, :].rearrange(
                                "(c p) q -> p c q", c=2))
                    for i in range(2):
                        kc = kc2 + i
                        for qb in range(NQB):
                            ps_t, ex_t = {}, {}
                            for h in heads:
                                po = (h % 2) * 64
                                ps_t[h] = psp.tile([128, 512], F32, tag="big",
                                                   name=f"ps{b}_{h}_{kc}_{qb}")
                                mm(ps_t[h][:, :],
                                   hk_sb[hc][po:po + 64,
                                             kc * 128:(kc + 1) * 128],
                                   hq_sb[hc][po:po + 64,
                                             qb * 512:(qb + 1) * 512],
                                   start=True, stop=USE_IDENT_ADD is False)
                            for h in heads:
                                if USE_IDENT_ADD:
                                    mm(ps_t[h][:, :], ident[:, :],
                                       pbm_t[h][:, i * LQ + qb * 512:
                                                i * LQ + (qb + 1) * 512],
                                       start=False, stop=True)
                            for h in heads:
                                ex_t[h] = tmpp.tile([128, 512], dt_v, tag="exp",
                                                    name=f"ex{b}_{h}_{kc}_{qb}")
                                if USE_IDENT_ADD:
                                    nc.scalar.activation(ex_t[h][:, :],
                                                         ps_t[h][:, :], Exp)
                                else:
                                    tmq = tmpp.tile([128, 512], F32, tag="tmq",
                                                    name=f"tq{b}_{h}_{kc}_{qb}")
                                    nc.vector.tensor_add(
                                        tmq[:, :], ps_t[h][:, :],
                                        pbm_t[h][:, i * LQ + qb * 512:
                                                 i * LQ + (qb + 1) * 512])
                                    nc.scalar.activation(ex_t[h][:, :],
                                                         tmq[:, :], Exp)
                            for h in heads:
                                mm(pctx[(h, qb)][:, :],
                                   hvT[kc][:, h * (DH + 1):(h + 1) * (DH + 1)],
                                   ex_t[h][:, :],
                                   start=kc == 0, stop=kc == NKC - 1)
                # normalize: ctx[0:64] * (1 / ctx[64]); reciprocals of the
                # pair's 4 denominator rows batched into one DVE op
                for h in heads:
                    po = (h % 2) * 64
                    for qb in range(NQB):
                        # 1/x as exp(-ln(x)) on ACT: x in [1, 1e21], and
                        # ACT Ln/Exp keep ~1e-6 rel err -- far cheaper than
                        # DVE reciprocal (4us per row)
                        rl = tmpp.tile([1, 512], F32, tag="rc",
                                       name=f"rl{b}_{h}_{qb}")
                        nc.scalar.activation(rl[:, :],
                                             pctx[(h, qb)][DH:DH + 1, :], Ln)
                        rc = tmpp.tile([1, 512], F32, tag="rc",
                                       name=f"rc{b}_{h}_{qb}")
                        nc.scalar.activation(rc[:, :], rl[:, :], Exp,
                                             scale=-1.0)
                        bc = tmpp.tile([64, 512], F32, tag="bc",
                                       name=f"bc{b}_{h}_{qb}")
                        nc.gpsimd.partition_broadcast(bc[:, :], rc[:, :])
                        nc.vector.tensor_mul(
                            ctxn[hc][po:po + 64, qb * 512:(qb + 1) * 512],
                            pctx[(h, qb)][0:DH, :], bc[:, :])

            # --- output projection ----------------------------------------
            if b == 0:
                load_wo()
            for oc in range(NOC):
                osb = tmpp.tile([128, LQ], dt_out, tag="osb",
                                name=f"osb{b}_{oc}")
                for qb in range(NQB):
                    po_t = psp.tile([128, 512], F32, tag="big",
                                    name=f"po{b}_{oc}_{qb}")
                    for c in range(NMC):
                        mm(po_t[:, :],
                           wo_sb[c][:, oc * 128:(oc + 1) * 128],
                           ctxn[c][:, qb * 512:(qb + 1) * 512],
                           start=c == 0, stop=c == NMC - 1)
                    nc.scalar.copy(osb[:, qb * 512:(qb + 1) * 512], po_t[:, :])
                nc.sync.dma_start(out_a[b, oc * 128:(oc + 1) * 128, :],
                                  osb[:, :])

    nc.compile()
    return nc


_NC_CACHE = None


def _get_nc():
    global _NC_CACHE
    if _NC_CACHE is None:
        _NC_CACHE = build_nc()
    return _NC_CACHE


def make_in_maps(hidden_q, hidden_kv, mask, position_bias, Wq, Wk, Wv, Wo):
    np_hid = _NP[CFG["dt_hid"]]
    np_w = _NP[CFG["dt_w"]]
    np_wo = _NP[CFG["dt_wo"]]
    np_pbm = _NP[CFG["dt_pbm"]]
    hidden_q = np.asarray(hidden_q, np.float32)
    hidden_kv = np.asarray(hidden_kv, np.float32)
    mask = np.asarray(mask)
    position_bias = np.asarray(position_bias, np.float32)
    Wq, Wk, Wv, Wo = (np.asarray(w, np.float32) for w in (Wq, Wk, Wv, Wo))

    maskb = np.where(mask != 0, np.float32(0), np.float32(NEG))  # [B, LK, LQ]
    hq = hidden_q.astype(np_hid)
    hkv = hidden_kv.astype(np_hid)
    ident = np.eye(128, dtype=np_pbm)
    in_maps = []
    for c in range(NCORES):
        hs = slice(c * HC, (c + 1) * HC)
        rs = slice(c * MR, (c + 1) * MR)
        pbm = (position_bias[hs][None] + maskb[:, None]).astype(np_pbm)
        in_maps.append({
            "hq": hq,
            "hkv": hkv,
            "pbm": pbm,
            "wqt": np.ascontiguousarray(Wq[rs].T).astype(np_w),
            "wkt": np.ascontiguousarray(Wk[rs].T).astype(np_w),
            "wvt": np.ascontiguousarray(Wv[rs].T).astype(np_w),
            "wot": np.ascontiguousarray(Wo[:, rs].T).astype(np_wo),
            "ident": ident,
        })
    return in_maps


def run(in_maps, trace=False):
    nc = _get_nc()
    return run_bass_kernel_spmd(nc, in_maps, core_ids=list(range(NCORES)),
                                trace=trace)


def kernel(hidden_q, hidden_kv, mask, position_bias, Wq, Wk, Wv, Wo):
    in_maps = make_in_maps(hidden_q, hidden_kv, mask, position_bias,
                           Wq, Wk, Wv, Wo)
    res = run(in_maps, trace=False)
    acc = np.zeros((B, D, LQ), np.float32)
    for r in res.results:
        acc += np.asarray(r["out"], dtype=np.float32)
    return acc


# revision 25
# speedup vs baseline: 1.2062x; 1.0474x over previous
"""Distributed sparse-attention kernel for 8 TRN2 NeuronCores.

Sharding: Megatron-style head parallelism. Core c owns heads [4c, 4c+4):
Wq/Wk/Wv column-parallel (rows of the [H*DH, D] weights), Wo row-parallel
(columns of [D, H*DH]). Each core computes a partial output
out_c = Wo_c @ ctx_c over its heads; the host sums the 8 partials.

Device layout choices (no on-chip transposes anywhere):
  h_q, h_k   : [dh, len] fp16  (proj psum M=dh-chunk, N=len)
  h_vT       : [k, dh+1] bf16  (proj psum M=k-chunk, N=dh; ones column
               appended so the ctx matmul also emits the softmax denom)
  scores     : [k, q] psum; pb+mask bias accumulated into the same psum
               bank by an identity-matmul (out += I.T @ pbm), exp on ACT
               straight from psum. No max-subtraction: scores are O(50)
               so exp fits fp32/bf16 range, masked entries are -1e30 ->
               exp underflows to exact 0.
  ctx        : psum [65, q] f32; row 64 = sum_k exp  (denominator)
  normalize  : reciprocal batched per head-pair on DVE + gpsimd
               partition_broadcast + DVE mul -> ctxn bf16

Precision: Q/K path (projections + scores) in fp16 — scores feed exp(),
so absolute score error must stay ~1e-2; fp16 keeps it ~5e-3 while bf16
would give ~5e-2. V/ctx/output path errors only enter linearly, so bf16
is fine there (exp values overflow fp16 range, hence bf16 anyway).
V-projection packs two k-chunk accumulations per PSUM bank: the first
matmul's start=True clears has_written for the whole bank, the partner
k-chunk then starts with start=False and overwrites-where-unset.
(Known-broken on HW, avoid: reciprocal_approx_fast — returns garbage.)
"""

import sys

for _p in ("/opt/trn_rl_repo",):
    if _p not in sys.path:
        sys.path.insert(0, _p)

from contextlib import ExitStack

import numpy as np
import ml_dtypes

import concourse.bass as bass
import concourse.mybir as mybir
import concourse.tile as tile
from concourse import bacc
from concourse.bass_utils import run_bass_kernel_spmd

B, D, H, DH, LQ, LK = 2, 2048, 32, 64, 1024, 1024
NCORES = 8
HC = H // NCORES          # heads per core = 4
MR = HC * DH              # per-core model rows = 256
NEG = -1e30

DC = D // 128             # 16 d-chunks
NKC = LK // 128           # 8 k-chunks
NQB = LQ // 512           # 2 q blocks
NMC = MR // 128           # 2 dh-chunks
NOC = D // 128            # 16 output-row chunks

F32 = mybir.dt.float32
F32R = mybir.dt.float32r
F16 = mybir.dt.float16
BF16 = mybir.dt.bfloat16

USE_IDENT_ADD = True

CFG = dict(
    dt_hid=F16,    # HBM hidden_q / hidden_kv (Q/K-path matmul operand)
    dt_w=F16,      # HBM Wq/Wk/Wv
    dt_wo=BF16,    # HBM Wo
    dt_pbm=BF16,   # HBM combined position_bias + mask bias
    dt_out=BF16,   # HBM partial output
    dt_qk=F16,     # SBUF h_q / h_k
    dt_v=BF16,     # SBUF h_vT / exp / ctxn
)

_NP = {F32: np.float32, F32R: np.float32, F16: np.float16,
       BF16: ml_dtypes.bfloat16}


def build_nc():
    dt_hid, dt_w, dt_wo = CFG["dt_hid"], CFG["dt_w"], CFG["dt_wo"]
    dt_pbm, dt_out = CFG["dt_pbm"], CFG["dt_out"]
    dt_qk, dt_v = CFG["dt_qk"], CFG["dt_v"]

    nc = bacc.Bacc("TRN2", target_bir_lowering=False, debug=False,
                   num_devices=NCORES)
    hq_e = nc.declare_dram_parameter("hq", [B, D, LQ], dt_hid, False)
    hkv_e = nc.declare_dram_parameter("hkv", [B, D, LK], dt_hid, False)
    pbm_e = nc.declare_dram_parameter("pbm", [B, HC, LK, LQ], dt_pbm, False)
    wqt_e = nc.declare_dram_parameter("wqt", [D, MR], dt_w, False)
    wkt_e = nc.declare_dram_parameter("wkt", [D, MR], dt_w, False)
    wvt_e = nc.declare_dram_parameter("wvt", [D, MR], dt_w, False)
    wot_e = nc.declare_dram_parameter("wot", [MR, D], dt_wo, False)
    id_e = nc.declare_dram_parameter("ident", [128, 128], dt_pbm, False)
    out_e = nc.declare_dram_parameter("out", [B, D, LQ], dt_out, True)
    hq_a, hkv_a, pbm_a, out_a = hq_e.ap(), hkv_e.ap(), pbm_e.ap(), out_e.ap()

    mm = nc.tensor.matmul
    Exp = mybir.ActivationFunctionType.Exp
    Ln = mybir.ActivationFunctionType.Ln

    with tile.TileContext(nc) as tc, ExitStack() as ctx:
        wp = ctx.enter_context(tc.tile_pool(name="w", bufs=1))
        hidp = ctx.enter_context(tc.tile_pool(name="hid", bufs=3))
        sbp = ctx.enter_context(tc.tile_pool(name="sb", bufs=1))
        pbmp = ctx.enter_context(tc.tile_pool(name="pbm", bufs=3))
        tmpp = ctx.enter_context(tc.tile_pool(name="tmp", bufs=4))
        psp = ctx.enter_context(tc.tile_pool(name="ps", bufs=4, space="PSUM"))

        # --- persistent weights, 4 d-chunks per DMA ----------------------
        # wX_sb[dc] are AP views into [128, 4*MR] tiles loaded with the
        # rearranged [(4c p) m -> p (4c m)] pattern.
        def load_w4(nm, ap_):
            views = []
            for g in range(DC // 4):
                t = wp.tile([128, 4 * MR], dt_w, tag=f"{nm}{g}", name=f"{nm}{g}")
                src = ap_[g * 512:(g + 1) * 512, :].rearrange(
                    "(c p) m -> p c m", c=4)
                nc.sync.dma_start(t[:, :].rearrange("p (c m) -> p c m", c=4),
                                  src)
                for i in range(4):
                    views.append(t[:, i * MR:(i + 1) * MR])
            return views

        wk_sb = load_w4("wk", wkt_e.ap())
        wv_sb = load_w4("wv", wvt_e.ap())
        ident = wp.tile([128, 128], dt_pbm, tag="ident", name="ident")
        nc.sync.dma_start(ident[:, :], id_e.ap()[:, :])
        ones1 = wp.tile([128, 1], F32, tag="ones1", name="ones1")
        nc.gpsimd.memset(ones1[:, :], 1.0)

        wq_sb, wo_sb = [], []

        def load_wq():
            wq_sb.extend(load_w4("wq", wqt_e.ap()))

        def load_wo():
            for c in range(NMC):
                t = wp.tile([128, D], dt_wo, tag=f"wo{c}", name=f"wo{c}")
                nc.sync.dma_start(t[:, :], wot_e.ap()[c * 128:(c + 1) * 128, :])
                wo_sb.append(t)

        for b in range(B):
            # --- pass A: K proj (4 banks) + V proj 2 k-chunks/bank (4) ---
            pk = [psp.tile([128, 512], F32, tag="big", name=f"pk{b}_{i}")
                  for i in range(4)]
            pv = [psp.tile([128, 512], F32, tag="pv", name=f"pv{b}_{i}")
                  for i in range(4)]
            hvT = [sbp.tile([128, HC * (DH + 1)], dt_v, tag=f"hv{kc}",
                            name=f"hv{b}_{kc}") for kc in range(NKC)]
            hk_sb = [sbp.tile([128, LK], dt_qk, tag=f"hk{mc}",
                              name=f"hk{b}_{mc}") for mc in range(NMC)]
            hq_sb = [sbp.tile([128, LQ], dt_qk, tag=f"hqs{mc}",
                              name=f"hqs{b}_{mc}") for mc in range(NMC)]

            for dc2 in range(0, DC, 2):
                hkv_t = hidp.tile([128, 2 * LK], dt_hid, tag="hkv",
                                  name=f"hkvA{b}_{dc2}")
                nc.sync.dma_start(
                    hkv_t[:, :].rearrange("p (c q) -> p c q", c=2),
                    hkv_a[b, dc2 * 128:(dc2 + 2) * 128, :].rearrange(
                        "(c p) q -> p c q", c=2))
                for i in range(2):
                    dc = dc2 + i
                    hv = hkv_t[:, i * LK:(i + 1) * LK]
                    for mc in range(NMC):
                        for kb in range(2):
                            mm(pk[mc * 2 + kb][:, :],
                               wk_sb[dc][:, mc * 128:(mc + 1) * 128],
                               hv[:, kb * 512:(kb + 1) * 512],
                               start=dc == 0, stop=dc == DC - 1)
                    for kc in range(NKC):
                        # two k-chunks share a psum bank; the first matmul
                        # clears the bank, the last one closes the group
                        mm(pv[kc // 2][:, (kc % 2) * MR:(kc % 2) * MR + MR],
                           hv[:, kc * 128:(kc + 1) * 128],
                           wv_sb[dc][:, :],
                           start=(dc == 0 and kc % 2 == 0),
                           stop=(dc == DC - 1 and kc % 2 == 1))
            for mc in range(NMC):
                for kb in range(2):
                    nc.vector.tensor_copy(hk_sb[mc][:, kb * 512:(kb + 1) * 512],
                                          pk[mc * 2 + kb][:, :])
            for kc in range(NKC):
                src = pv[kc // 2][:, (kc % 2) * MR:(kc % 2) * MR + MR]
                for h in range(HC):
                    nc.vector.tensor_copy(
                        hvT[kc][:, h * (DH + 1):h * (DH + 1) + DH],
                        src[:, h * DH:(h + 1) * DH])
                    nc.vector.tensor_copy(
                        hvT[kc][:, h * (DH + 1) + DH:(h + 1) * (DH + 1)],
                        ones1[:, :])

            # --- pass B: Q proj ------------------------------------------
            if b == 0:
                load_wq()
            pq = [psp.tile([128, 512], F32, tag="big", name=f"pq{b}_{i}")
                  for i in range(4)]
            for dc2 in range(0, DC, 2):
                hq_t = hidp.tile([128, 2 * LQ], dt_hid, tag="hq",
                                 name=f"hqB{b}_{dc2}")
                nc.sync.dma_start(
                    hq_t[:, :].rearrange("p (c q) -> p c q", c=2),
                    hq_a[b, dc2 * 128:(dc2 + 2) * 128, :].rearrange(
                        "(c p) q -> p c q", c=2))
                for i in range(2):
                    dc = dc2 + i
                    hv = hq_t[:, i * LQ:(i + 1) * LQ]
                    for mc in range(NMC):
                        for qb in range(2):
                            mm(pq[mc * 2 + qb][:, :],
                               wq_sb[dc][:, mc * 128:(mc + 1) * 128],
                               hv[:, qb * 512:(qb + 1) * 512],
                               start=dc == 0, stop=dc == DC - 1)
            for mc in range(NMC):
                for qb in range(2):
                    nc.vector.tensor_copy(hq_sb[mc][:, qb * 512:(qb + 1) * 512],
                                          pq[mc * 2 + qb][:, :])

            # --- attention, head pairs ------------------------------------
            ctxn = [sbp.tile([128, LQ], dt_v, tag=f"ctxn{c}",
                             name=f"ctxn{b}_{c}") for c in range(NMC)]
            for hp in range(HC // 2):
                hc = hp
                heads = (2 * hp, 2 * hp + 1)
                pctx = {}
                for h in heads:
                    for qb in range(NQB):
                        pctx[(h, qb)] = psp.tile([DH + 1, 512], F32, tag="pv",
                                                 name=f"pctx{b}_{h}_{qb}")
                for kc2 in range(0, NKC, 2):
                    pbm_t = {}
                    for h in heads:
                        pbm_t[h] = pbmp.tile([128, 2 * LQ], dt_pbm, tag="pbm",
                                             name=f"pbm{b}_{h}_{kc2}")
                        nc.sync.dma_start(
                            pbm_t[h][:, :].rearrange("p (c q) -> p c q", c=2),
                            pbm_a[b, h, kc2 * 128:(kc2 + 2) * 128, :].rearrange(
                                "(c p) q -> p c q", c=2))
                    for i in range(2):
                        kc = kc2 + i
                        for qb in range(NQB):
                            ps_t, ex_t = {}, {}
                            for h in heads:
                                po = (h % 2) * 64
                                ps_t[h] = psp.tile([128, 512], F32, tag="big",
                                                   name=f"ps{b}_{h}_{kc}_{qb}")
                                mm(ps_t[h][:, :],
                                   hk_sb[hc][po:po + 64,
                                             kc * 128:(kc + 1) * 128],
                                   hq_sb[hc][po:po + 64,
                                             qb * 512:(qb + 1) * 512],
                                   start=True, stop=USE_IDENT_ADD is False)
                            for h in heads:
                                if USE_IDENT_ADD:
                                    mm(ps_t[h][:, :], ident[:, :],
                                       pbm_t[h][:, i * LQ + qb * 512:
                                                i * LQ + (qb + 1) * 512],
                                       start=False, stop=True)
                            for h in heads:
                                ex_t[h] = tmpp.tile([128, 512], dt_v, tag="exp",
                                                    name=f"ex{b}_{h}_{kc}_{qb}")
                                if USE_IDENT_ADD:
                                    nc.scalar.activation(ex_t[h][:, :],
                                                         ps_t[h][:, :], Exp)
                                else:
                                    tmq = tmpp.tile([128, 512], F32, tag="tmq",
                                                    name=f"tq{b}_{h}_{kc}_{qb}")
                                    nc.vector.tensor_add(
                                        tmq[:, :], ps_t[h][:, :],
                                        pbm_t[h][:, i * LQ + qb * 512:
                                                 i * LQ + (qb + 1) * 512])
                                    nc.scalar.activation(ex_t[h][:, :],
                                                         tmq[:, :], Exp)
                            for h in heads:
                                mm(pctx[(h, qb)][:, :],
                                   hvT[kc][:, h * (DH + 1):(h + 1) * (DH + 1)],
                                   ex_t[h][:, :],
                                   start=kc == 0, stop=kc == NKC - 1)
                # normalize: ctx[0:64] * (1 / ctx[64]); reciprocals of the
                # pair's 4 denominator rows batched into one DVE op
                # 1/x as exp(-ln(x)) on ACT: x in [1, 1e21], ~1e-6 rel
                # err, far cheaper than DVE reciprocal (4us per row).
                # Ln's and Exp's batched to avoid ACT function-table
                # reloads (~1.3us per Exp<->Ln switch).
                hqbs = [(h, qb) for h in heads for qb in range(NQB)]
                rl_t, rc_t = {}, {}
                for j, (h, qb) in enumerate(hqbs):
                    rl_t[j] = tmpp.tile([1, 512], F32, tag="rl",
                                        name=f"rl{b}_{h}_{qb}")
                    nc.scalar.activation(rl_t[j][:, :],
                                         pctx[(h, qb)][DH:DH + 1, :], Ln)
                for j, (h, qb) in enumerate(hqbs):
                    rc_t[j] = tmpp.tile([1, 512], F32, tag="rc",
                                        name=f"rc{b}_{h}_{qb}")
                    nc.scalar.activation(rc_t[j][:, :], rl_t[j][:, :], Exp,
                                         scale=-1.0)
                for j, (h, qb) in enumerate(hqbs):
                    po = (h % 2) * 64
                    bc = tmpp.tile([64, 512], F32, tag="bc",
                                   name=f"bc{b}_{h}_{qb}")
                    nc.gpsimd.partition_broadcast(bc[:, :], rc_t[j][:, :])
                    nc.vector.tensor_mul(
                        ctxn[hc][po:po + 64, qb * 512:(qb + 1) * 512],
                        pctx[(h, qb)][0:DH, :], bc[:, :])

            # --- output projection ----------------------------------------
            if b == 0:
                load_wo()
            for oc in range(NOC):
                osb = tmpp.tile([128, LQ], dt_out, tag="osb",
                                name=f"osb{b}_{oc}")
                for qb in range(NQB):
                    po_t = psp.tile([128, 512], F32, tag="big",
                                    name=f"po{b}_{oc}_{qb}")
                    for c in range(NMC):
                        mm(po_t[:, :],
                           wo_sb[c][:, oc * 128:(oc + 1) * 128],
                           ctxn[c][:, qb * 512:(qb + 1) * 512],
                           start=c == 0, stop=c == NMC - 1)
                    nc.scalar.copy(osb[:, qb * 512:(qb + 1) * 512], po_t[:, :])
                nc.sync.dma_start(out_a[b, oc * 128:(oc + 1) * 128, :],
                                  osb[:, :])

    nc.compile()
    return nc


_NC_CACHE = None


def _get_nc():
    global _NC_CACHE
    if _NC_CACHE is None:
        _NC_CACHE = build_nc()
    return _NC_CACHE


def make_in_maps(hidden_q, hidden_kv, mask, position_bias, Wq, Wk, Wv, Wo):
    np_hid = _NP[CFG["dt_hid"]]
    np_w = _NP[CFG["dt_w"]]
    np_wo = _NP[CFG["dt_wo"]]
    np_pbm = _NP[CFG["dt_pbm"]]
    hidden_q = np.asarray(hidden_q, np.float32)
    hidden_kv = np.asarray(hidden_kv, np.float32)
    mask = np.asarray(mask)
    position_bias = np.asarray(position_bias, np.float32)
    Wq, Wk, Wv, Wo = (np.asarray(w, np.float32) for w in (Wq, Wk, Wv, Wo))

    maskb = np.where(mask != 0, np.float32(0), np.float32(NEG))  # [B, LK, LQ]
    hq = hidden_q.astype(np_hid)
    hkv = hidden_kv.astype(np_hid)
    ident = np.eye(128, dtype=np_pbm)
    in_maps = []
    for c in range(NCORES):
        hs = slice(c * HC, (c + 1) * HC)
        rs = slice(c * MR, (c + 1) * MR)
        pbm = (position_bias[hs][None] + maskb[:, None]).astype(np_pbm)
        in_maps.append({
            "hq": hq,
            "hkv": hkv,
            "pbm": pbm,
            "wqt": np.ascontiguousarray(Wq[rs].T).astype(np_w),
            "wkt": np.ascontiguousarray(Wk[rs].T).astype(np_w),
            "wvt": np.ascontiguousarray(Wv[rs].T).astype(np_w),
            "wot": np.ascontiguousarray(Wo[:, rs].T).astype(np_wo),
            "ident": ident,
        })
    return in_maps


def run(in_maps, trace=False):
    nc = _get_nc()
    return run_bass_kernel_spmd(nc, in_maps, core_ids=list(range(NCORES)),
                                trace=trace)


def kernel(hidden_q, hidden_kv, mask, position_bias, Wq, Wk, Wv, Wo):
    in_maps = make_in_maps(hidden_q, hidden_kv, mask, position_bias,
                           Wq, Wk, Wv, Wo)
    res = run(in_maps, trace=False)
    acc = np.zeros((B, D, LQ), np.float32)
    for r in res.results:
        acc += np.asarray(r["out"], dtype=np.float32)
    return acc
